# revision 12
# baseline (speedup 1.0000x reference)
"""Expert-parallel MoE SwiGLU kernel for 8 Trainium2 NeuronCores.

Strategy: expert parallelism with host-side dispatch/combine. Each of the
8 cores owns one expert's weights. The host routes tokens by expert_idx,
packs each expert's tokens as a transposed [D, W] panel (features on
partitions so no on-chip transposes are needed anywhere), and each core
runs a dense SwiGLU FFN:  yT = w_down.T-blocks @ (silu(wg.T@xT) * (wu.T@xT)).
Matmul operands stream as fp16 (fp32 PSUM accumulation; ~6e-4 max
relative error vs the fp32 reference), halving the weight traffic that
dominates this memory-bound kernel.

Schedule (v3): baseline f-major pipeline over 4 f-groups of 1024 with
the previous group's down-projection interleaved, plus three opening
fixes that remove the long HAM-warmup dummy burst and the DMA-gated
gaps of the original opening:
 - x is shipped as ONE [128, 8W] DMA (host packs d-chunks side by side)
   so the sync ring spends 0.65us on it instead of 5us of issue time.
 - group 0's first 512 f-columns run d-OUTER: 4 gate + 4 up PSUM chains
   are fed one d-chunk at a time, so real matmuls start as soon as the
   first [128,1024] weight tile lands (~9us) instead of after all 8
   gate tiles (~12us). wu group-0 tiles stream on the scalar ring,
   which is idle before the first activations.
 - only ~6 warmup dummies (to pre-trip the HAM activity window during
   the unavoidable DMA lead-in) instead of 16.
Tail: final y write-outs alternate sync/scalar rings.
"""

import numpy as np
from contextlib import ExitStack

D_MODEL = 1024
D_FF = 4096
N_EXPERTS = 8
N_CORES = 8

_ND = D_MODEL // 128  # 8 contraction chunks over d_model
_NF = D_FF // 128     # 32 f chunks

_nc_cache = {}

import os as _os
_CDT = _os.environ.get("MOE_KERNEL_DTYPE", "float16")

_FSG = 1024           # f columns per gate/up weight streaming tile
_NFSG = D_FF // _FSG  # 4 groups
_FTG = _FSG // 128    # 8 f-tiles per group


def _np_cdt():
    if _CDT == "float16":
        return np.float16
    if _CDT == "bfloat16":
        import ml_dtypes
        return ml_dtypes.bfloat16
    return np.float32


def _build_nc(W: int):
    """Build + schedule the per-core Bass program for token capacity W."""
    import concourse.bacc as bacc
    import concourse.tile as tile
    from concourse import mybir

    f32 = mybir.dt.float32
    f32r = getattr(mybir.dt, _CDT)

    nc = bacc.Bacc("TRN2", target_bir_lowering=False, debug=False,
                   num_devices=N_CORES)
    Wp = (W + 31) // 32 * 32   # 64B-aligned d-chunk slots
    xt = nc.dram_tensor("xt", [128, _ND * Wp], f32r, kind="ExternalInput").ap()
    wg = nc.dram_tensor("wg", [_NFSG, _ND, 128, _FSG], f32r,
                        kind="ExternalInput").ap()
    wu = nc.dram_tensor("wu", [_NFSG, _ND, 128, _FSG], f32r,
                        kind="ExternalInput").ap()
    wd = nc.dram_tensor("wd", [D_FF, D_MODEL], f32r, kind="ExternalInput").ap()
    yt = nc.dram_tensor("yt", [D_MODEL, W], f32, kind="ExternalOutput").ap()

    _ctr = [0]

    def _nm():
        _ctr[0] += 1
        return _ctr[0]

    with tile.TileContext(nc) as tc, ExitStack() as ctx:
        xpool = ctx.enter_context(tc.tile_pool(name="x", bufs=1))
        wgp = ctx.enter_context(tc.tile_pool(name="wgp", bufs=3))
        wup = ctx.enter_context(tc.tile_pool(name="wup", bufs=3))
        wdp = ctx.enter_context(tc.tile_pool(name="wdp", bufs=2))
        tp = ctx.enter_context(tc.tile_pool(name="tp", bufs=2))
        gap = ctx.enter_context(tc.tile_pool(name="gap", bufs=3))
        yp = ctx.enter_context(tc.tile_pool(name="yp", bufs=1))
        pp = ctx.enter_context(tc.tile_pool(name="pp", bufs=8, space="PSUM"))

        # All input activations in TWO DMAs: host packs xT d-chunks side
        # by side as [128, 8*Wp] (64B-aligned slots so the PE moving-
        # operand slices stay aligned); xts[d] are column slices.
        x_all = xpool.tile([128, _ND * Wp], f32r, tag="xall", name="x_all")
        half = _ND // 2 * Wp
        xts = [x_all[:, d * Wp:d * Wp + W] for d in range(_ND)]

        y_acc = [yp.tile([128, W], f32, tag=f"y{d}", name=f"y_acc{d}")
                 for d in range(_ND)]

        # Warmup scratch: dummy matmuls pre-trip the HAM activity window
        # while x / the first weight tiles are still in flight.
        scr_w = xpool.tile([128, 128], f32r, tag="scrw", name="scr_w")
        scr_x = xpool.tile([128, W], f32r, tag="scrx", name="scr_x")
        nc.vector.memset(scr_w[:], 0.0)
        nc.vector.memset(scr_x[:], 0.0)
        scr_p = [pp.tile([128, W], f32, tag="ps", name=f"scr_p{i}")
                 for i in range(2)]
        def dummies(n):
            for i in range(n):
                _ctr[0] += 1
                nc.tensor.matmul(scr_p[_ctr[0] % 2][:], scr_w[:], scr_x[:],
                                 start=True, stop=True)

        dummies(8)

        # Group 0 weight tiles: wg on sync ring, wu on the (idle) scalar
        # ring, in d order so the d-outer opening consumes them as they
        # land.
        # Group-0 weight tiles, ordered by first-use: the d-outer opening
        # consumes (wg0[d], wu0[d]) pairs in d order, so each ring leads
        # with exactly the d-step-0 tiles and x rides along in small
        # chunks scheduled just before the d-steps that read them.
        wg0 = [wgp.tile([128, _FSG], f32r, tag=f"wg{d}", name=f"wg0_{d}")
               for d in range(_ND)]
        wu0 = [wup.tile([128, _FSG], f32r, tag=f"wu{d}", name=f"wu0_{d}")
               for d in range(_ND)]
        nc.sync.dma_start(x_all[:, 0:Wp], xt[:, 0:Wp])            # x d0
        nc.sync.dma_start(wg0[0][:], wg[0, 0])
        nc.sync.dma_start(wg0[1][:], wg[0, 1])
        nc.sync.dma_start(x_all[:, Wp:4 * Wp], xt[:, Wp:4 * Wp])  # x d1-3
        for d in range(2, _ND):
            nc.sync.dma_start(wg0[d][:], wg[0, d])
        nc.scalar.dma_start(wu0[0][:], wu[0, 0])
        nc.scalar.dma_start(wu0[1][:], wu[0, 1])
        nc.scalar.dma_start(x_all[:, 4 * Wp:], xt[:, 4 * Wp:])    # x d4-7
        for d in range(2, _ND):
            nc.scalar.dma_start(wu0[d][:], wu[0, d])

        def _swiglu(psg, psu, ft):
            g_act = gap.tile([128, W], f32, tag="gact", name=f"ga_{_nm()}")
            nc.scalar.activation(g_act[:], psg[:],
                                 mybir.ActivationFunctionType.Silu)
            t_t = tp.tile([128, W], f32r, tag=f"t{ft}", name=f"t_{_nm()}")
            nc.vector.tensor_mul(t_t[:], g_act[:], psu[:])
            return t_t

        # Down-projection of the PREVIOUS f group is interleaved between
        # this group's matmul bursts.
        def emit_down(fsg, t_tiles, wd_tiles, dts, last=False):
            for dt in dts:
                pdt = pp.tile([128, W], f32, tag="ps", name=f"pd_{_nm()}")
                for ft in range(_FTG):
                    nc.tensor.matmul(
                        pdt[:],
                        wd_tiles[ft][:, dt * 128:(dt + 1) * 128],
                        t_tiles[ft][:],
                        start=(ft == 0), stop=(ft == _FTG - 1))
                if fsg == 0:
                    nc.vector.tensor_copy(y_acc[dt][:], pdt[:])
                else:
                    nc.vector.tensor_add(y_acc[dt][:], y_acc[dt][:], pdt[:])
                if last:
                    eng = nc.sync if dt % 2 == 0 else nc.scalar
                    eng.dma_start(yt[dt * 128:(dt + 1) * 128, :],
                                  y_acc[dt][:])

        # ---------------- group 0 ----------------
        # Phase A: f-tiles 0..3 d-OUTER — chains fill as weight tiles
        # arrive; PE is busy from the first tile instead of waiting for
        # all eight.
        psg0 = [pp.tile([128, W], f32, tag="ps", name=f"pg0_{i}")
                for i in range(3)]
        psu0 = [pp.tile([128, W], f32, tag="ps", name=f"pu0_{i}")
                for i in range(3)]
        for d in range(_ND):
            for ft in range(3):
                nc.tensor.matmul(
                    psg0[ft][:], wg0[d][:, ft * 128:(ft + 1) * 128],
                    xts[d], start=(d == 0), stop=(d == _ND - 1))
                nc.tensor.matmul(
                    psu0[ft][:], wu0[d][:, ft * 128:(ft + 1) * 128],
                    xts[d], start=(d == 0), stop=(d == _ND - 1))
            if 1 <= d <= 3:
                # keep the HAM activity window busy through the
                # supply-limited cold steps so the clock warms early
                dummies(2)
        t_g0 = [_swiglu(psg0[ft], psu0[ft], ft) for ft in range(3)]

        # wd tiles for group 0 (consumed by downs during group 1)
        wd_prev = []
        for ft in range(_FTG):
            wd_t = wdp.tile([128, D_MODEL], f32r, tag=f"wd{ft}",
                            name=f"wd0_{ft}")
            nc.sync.dma_start(wd_t[:], wd[ft * 128:(ft + 1) * 128, :])
            wd_prev.append(wd_t)

        # Phase B: f-tiles 3..7 f-major (all tiles resident by now)
        for ft in range(3, _FTG):
            psg = pp.tile([128, W], f32, tag="ps", name=f"pg_{_nm()}")
            for d in range(_ND):
                nc.tensor.matmul(
                    psg[:], wg0[d][:, ft * 128:(ft + 1) * 128], xts[d],
                    start=(d == 0), stop=(d == _ND - 1))
            psu = pp.tile([128, W], f32, tag="ps", name=f"pu_{_nm()}")
            for d in range(_ND):
                nc.tensor.matmul(
                    psu[:], wu0[d][:, ft * 128:(ft + 1) * 128], xts[d],
                    start=(d == 0), stop=(d == _ND - 1))
            t_g0.append(_swiglu(psg, psu, ft))

        prev = (0, t_g0, wd_prev)

        # ---------------- groups 1..3 ----------------
        for fsg in range(1, _NFSG):
            wg_t, wu_t = [], []
            for d in range(_ND):
                g_t = wgp.tile([128, _FSG], f32r, tag=f"wg{d}",
                               name=f"wg{fsg}_{d}")
                nc.sync.dma_start(g_t[:], wg[fsg, d])
                wg_t.append(g_t)
                u_t = wup.tile([128, _FSG], f32r, tag=f"wu{d}",
                               name=f"wu{fsg}_{d}")
                nc.sync.dma_start(u_t[:], wu[fsg, d])
                wu_t.append(u_t)

            t_tiles = []
            wd_tiles = []
            for ft in range(_FTG):
                fc = fsg * _FTG + ft
                wd_t = wdp.tile([128, D_MODEL], f32r, tag=f"wd{ft}",
                                name=f"wd{fsg}_{ft}")
                nc.sync.dma_start(wd_t[:], wd[fc * 128:(fc + 1) * 128, :])
                wd_tiles.append(wd_t)
                psg = pp.tile([128, W], f32, tag="ps", name=f"pg_{_nm()}")
                for d in range(_ND):
                    nc.tensor.matmul(
                        psg[:], wg_t[d][:, ft * 128:(ft + 1) * 128],
                        xts[d], start=(d == 0), stop=(d == _ND - 1))
                psu = pp.tile([128, W], f32, tag="ps", name=f"pu_{_nm()}")
                for d in range(_ND):
                    nc.tensor.matmul(
                        psu[:], wu_t[d][:, ft * 128:(ft + 1) * 128],
                        xts[d], start=(d == 0), stop=(d == _ND - 1))
                t_tiles.append(_swiglu(psg, psu, ft))
                emit_down(prev[0], prev[1], prev[2], (ft,))
            prev = (fsg, t_tiles, wd_tiles)

        # ---------------- tail: downs of the last group ----------------
        emit_down(prev[0], prev[1], prev[2], range(_ND), last=True)

    nc.compile()
    return nc


def _pack_gu(w):
    # [D, F] -> [NFSG, ND, 128, FSG] so each streamed tile is contiguous
    w = np.asarray(w).astype(_np_cdt())
    return np.ascontiguousarray(
        w.reshape(_ND, 128, _NFSG, _FSG).transpose(2, 0, 1, 3))


def _run_one(W, tok_lists, x_flat, packed_w, out_flat):
    from concourse.bass_utils import run_bass_kernel_spmd

    if W not in _nc_cache:
        _nc_cache[W] = _build_nc(W)
    nc = _nc_cache[W]

    D = x_flat.shape[1]
    in_maps = []
    for e in range(N_EXPERTS):
        toks = tok_lists[e]
        # xT packed as [128, ND*Wp]: d-chunk d at cols [d*Wp, d*Wp+W)
        Wp = (W + 31) // 32 * 32
        xt_e = np.zeros((128, _ND * Wp), dtype=_np_cdt())
        xe = x_flat[toks].T.astype(_np_cdt())          # [D, n]
        n = len(toks)
        for d in range(_ND):
            xt_e[:, d * Wp:d * Wp + n] = xe[d * 128:(d + 1) * 128, :]
        in_maps.append({
            "xt": xt_e,
            "wg": packed_w[e][0],
            "wu": packed_w[e][1],
            "wd": packed_w[e][2],
        })

    res = None
    for attempt in range(3):
        try:
            res = run_bass_kernel_spmd(nc, in_maps,
                                       core_ids=list(range(N_CORES)))
            break
        except Exception:
            if attempt == 2:
                raise
            import time
            time.sleep(3.0)
            try:
                import jax
                jax.clear_caches()
                jax.clear_backends()
            except Exception:
                pass
    for e in range(N_EXPERTS):
        toks = tok_lists[e]
        out_flat[toks] = res.results[e]["yt"][:, :len(toks)].T


def kernel(x, expert_idx, w_gate, w_up, w_down):
    x = np.asarray(x, dtype=np.float32)
    idx = np.asarray(expert_idx).astype(np.int64)
    B, S, D = x.shape
    T = B * S
    x_flat = np.ascontiguousarray(x.reshape(T, D))
    idx_flat = idx.reshape(T)

    packed_w = [
        (_pack_gu(w_gate[e]), _pack_gu(w_up[e]),
         np.ascontiguousarray(np.asarray(w_down[e]).astype(_np_cdt())))
        for e in range(N_EXPERTS)
    ]

    tok_lists = [np.nonzero(idx_flat == e)[0] for e in range(N_EXPERTS)]
    cap = max(1, max(len(t) for t in tok_lists))
    out_flat = np.zeros((T, D), dtype=np.float32)

    if cap <= 512:
        # normal path: one SPMD run, capacity = max expert load (floor 256
        # keeps DMA partition lines >= 512B)
        W = max(256, cap)
        _run_one(W, tok_lists, x_flat, packed_w, out_flat)
    else:
        # fallback for extreme routing imbalance: process tokens in
        # rounds of <=512 per expert, reusing one compiled W=512 program
        rounds = -(-cap // 512)
        for r in range(rounds):
            round_lists = [t[r * 512:(r + 1) * 512] for t in tok_lists]
            _run_one(512, round_lists, x_flat, packed_w, out_flat)

    return out_flat.reshape(B, S, D)
